# revision 2
# baseline (speedup 1.0000x reference)
"""Trainium2 Bass kernel for prefix-KV multi-head attention.

Reference computation (per batch):
    qkv = x @ w_qkv -> q,k,v heads; k/v get a 16-token prefix (pk, pv)
    attn = softmax(q @ k^T * D^-0.5); out = (attn @ v) @ w_proj + b_proj

Sharding: data-parallel over B across 8 NeuronCores (2 batches per core).
Everything on-chip is computed in "feature-major" (transposed) layouts so
that every matmul contraction lands on the partition axis with zero
runtime data reshuffling:

  x^T   [C, n]   via PE transposes of x
  q^T/k^T [f, n] = w_qkv-tile (stationary) x x^T (moving), fp32r
  scores^T [m, n] per (head, m-tile): lhsT = k^T slice [64, 128]
  E^T = exp(scale * scores^T)  (ACT, reading PSUM directly)
  attn@v: lhsT = [v_h | 64 ones-columns] [m-tile, 128] -> psum rows 0:64 =
      unnormalized out^T, rows 64:128 = softmax denominator REPLICATED,
      accumulated over the 9 m-tiles (m-tile 0 = zero-padded prefix).
  out2^T = psum[0:64] * reciprocal(psum[64:128])
  final^T [c', n] = w_proj-tile (stationary) x out2^T (moving) + b (per-
      partition bias); host transposes the [C, N] result back to [N, C].

fp32r (rounded fp32) runs the PE at bf16 speed for moving dims >= 256
with ~1.2e-4 relative rounding error.

This file is self-contained: it monkeypatches two workarounds for the
walrus build in this container (1-sync-wait-per-instruction cap).
"""

import json
import os
import sys

for _p in ("/opt/trn_rl_repo", os.path.expanduser("~/.axon_site/_ro/trn_rl_repo")):
    if os.path.isdir(_p) and _p not in sys.path:
        sys.path.insert(0, _p)

import numpy as np

import concourse.bass as bass
import concourse.tile as tile
from concourse import mybir
from concourse.bass_utils import run_bass_kernel_spmd
from concourse.vector_clock import ScopedClock
from concourse.masks import make_identity

F32 = mybir.dt.float32
F32R = mybir.dt.float32r
BF16 = mybir.dt.bfloat16
AF = mybir.ActivationFunctionType

# ---------------------------------------------------------------------------
# Workaround: this container's walrus supports at most ONE sync wait per
# instruction.  (a) split the TileContext-exit drain's waits onto single-wait
# NOPs; (b) at BIR-JSON serialization time, hoist extra waits from any
# instruction onto same-engine NOPs placed immediately before it.
# ---------------------------------------------------------------------------

def _patched_drain_and_barrier(self, tick_clock, wait_clock):
    drain_inst = self.nc.sync.drain()
    wait_clock.add_sem_waits(
        drain_inst.ins, ScopedClock({None: tick_clock.global_clock})
    )
    si = drain_inst.ins.sync_info
    waits = list(si.on_wait) if si is not None and si.on_wait else []
    if len(waits) > 1:
        si.on_wait = waits[:1]
        for w in waits[1:]:
            nop = self.nc.sync.nop(hint="drain_wait_split", nofuse=True)
            nsi = nop.ins.sync_info
            if nsi is None:
                nop.ins.sync_info = mybir.SyncInfo(on_wait=[w], on_update=[])
            else:
                nsi.on_wait = list(nsi.on_wait or []) + [w]
    self.nc.all_engine_barrier()
    assert self.sems is not None
    popped = self.nc._tile_sem_poison_stack.pop()
    assert popped is self._sem_poison
    self.nc.clear_and_free_semaphores(list(self.sems.allocated().values()))
    self.nc.all_engine_barrier()


tile.TileContext._drain_and_barrier = _patched_drain_and_barrier


def _split_multi_waits(bir):
    for fn in bir["functions"]:
        for bb in fn["blocks"]:
            new_insts = []
            for inst in bb["instructions"]:
                si = inst.get("sync_info")
                ow = (si or {}).get("on_wait") or []
                if len(ow) > 1:
                    for i, w in enumerate(ow[:-1]):
                        new_insts.append({
                            "debug": inst.get("debug", 0),
                            "engine": inst["engine"],
                            "ins": [], "outs": [],
                            "name": f"{inst['name']}.wsplit{i}",
                            "opcode": "NoOp",
                            "sync_info": {"on_wait": [w], "on_update": []},
                        })
                    si["on_wait"] = [ow[-1]]
                new_insts.append(inst)
            bb["instructions"] = new_insts
    return bir


_orig_to_json_bytes = bass.Bass.to_json_bytes


def _patched_to_json_bytes(self):
    d = json.loads(_orig_to_json_bytes(self))
    _split_multi_waits(d)
    return json.dumps(d).encode()


bass.Bass.to_json_bytes = _patched_to_json_bytes

# ---------------------------------------------------------------------------
# Problem constants (hardcoded per the task contract)
# ---------------------------------------------------------------------------

B, N, C, H, P = 16, 1024, 1024, 16, 16
D = C // H                      # 64
SCALE = float(D) ** -0.5        # 0.125
N_CORES = 8
B_PC = B // N_CORES             # 2 batches per core
NT = N // 128                   # 8 token tiles
CT = C // 128                   # 8 feature tiles
MT = NT + 1                     # 9 m-tiles: tile 0 = prefix (16 valid rows)
HPAIRS = H // 2                 # 8 head pairs (2 heads per 128-row f-tile)


def build_nc(repeat: int = 1) -> bass.Bass:
    nc = bass.Bass()

    x_d = nc.declare_dram_parameter("x", [B_PC, N, C], F32, isOutput=False)
    pk_d = nc.declare_dram_parameter("pk", [B_PC, P, C], F32, isOutput=False)
    pv_d = nc.declare_dram_parameter("pv", [B_PC, P, C], F32, isOutput=False)
    wqkv_d = nc.declare_dram_parameter("w_qkv", [C, 3 * C], F32, isOutput=False)
    wproj_d = nc.declare_dram_parameter("w_proj", [C, C], F32, isOutput=False)
    bias_d = nc.declare_dram_parameter("b_proj", [C], F32, isOutput=False)
    # output is stored TRANSPOSED per batch: [C, N]; host transposes back
    outT_d = nc.declare_dram_parameter("outT", [B_PC, C, N], F32, isOutput=True)
    # internal DRAM scratch: q^T spilled per batch during phase A
    qsp_d = nc.dram_tensor("q_spill", [B_PC, CT, 128, N], BF16)

    with tile.TileContext(nc) as tc:
        with tc.tile_pool(name="cons", bufs=1) as cons, \
             tc.tile_pool(name="xload", bufs=3) as xload, \
             tc.tile_pool(name="big", bufs=1) as big_pool, \
             tc.tile_pool(name="kT", bufs=1) as kT_pool, \
             tc.tile_pool(name="vx", bufs=1) as vx_pool, \
             tc.tile_pool(name="qp", bufs=3) as q_pool, \
             tc.tile_pool(name="eT", bufs=4) as e_pool, \
             tc.tile_pool(name="rb", bufs=2) as rb_pool, \
             tc.tile_pool(name="osb", bufs=3) as o_pool, \
             tc.tile_pool(name="wq", bufs=3) as w_pool, \
             tc.tile_pool(name="psA", bufs=2, space="PSUM") as psA, \
             tc.tile_pool(name="psB", bufs=2, space="PSUM") as psB:

            ident = cons.tile([128, 128], F32, tag="ident")
            make_identity(nc, ident[:])
            # bias in per-partition layout: bias_col[p, cft] = b_proj[cft*128+p]
            bias_col = cons.tile([128, CT], F32, tag="bias")
            nc.sync.dma_start(
                out=bias_col[:],
                in_=bias_d[:].rearrange("(a b) -> b a", b=128),
            )

            for _rep in range(repeat):
              for b in range(B_PC):
                  # ---------------- phase 1: x^T ----------------
                  xT = big_pool.tile([128, CT, N], BF16, tag="big", name=f"xT_{_rep}_{b}")
                  for nt in range(NT):
                      xl = xload.tile([128, C], F32, tag="xl")
                      nc.sync.dma_start(
                          out=xl[:], in_=x_d[b, nt * 128:(nt + 1) * 128, :]
                      )
                      ps_t = psA.tile([128, N], F32, tag="psA")
                      for ct in range(CT):
                          nc.tensor.transpose(
                              ps_t[:, ct * 128:(ct + 1) * 128],
                              xl[:, ct * 128:(ct + 1) * 128],
                              ident[:],
                          )
                      nc.vector.tensor_copy(
                          xT[:, :, nt * 128:(nt + 1) * 128],
                          ps_t[:].rearrange("p (a c) -> p a c", c=128),
                      )

                  # ---------------- phase 2: k^T (f-tiles), prefix, pads -----
                  kT = kT_pool.tile([128, CT, MT * 128], BF16, tag="kT")
                  # zero the prefix-tile padding columns 16..128 (avoid inf/NaN)
                  nc.vector.memset(kT[:, :, P:128], 0.0)

                  # prefix keys: pk [16, C] -> pk^T columns 0..16 of each f-tile
                  pkl = xload.tile([128, C], F32, tag="xl")
                  nc.sync.dma_start(out=pkl[0:P, :], in_=pk_d[b, :, :])
                  ps_pk = psA.tile([128, N], F32, tag="psA")
                  for ct in range(CT):
                      nc.tensor.transpose(
                          ps_pk[:, ct * 128:ct * 128 + P],
                          pkl[0:P, ct * 128:(ct + 1) * 128],
                          ident[0:P, 0:P],
                      )
                  nc.vector.tensor_copy(
                      kT[:, :, 0:P],
                      ps_pk[:].rearrange("p (a c) -> p a c", c=128)[:, :, 0:P],
                  )

                  for ftp in range(CT // 2):  # k feature tiles, 2 per pass
                      ps_ks = [
                          psA.tile([128, N], F32, tag="psA",
                                   name=f"ps_k_{_rep}_{b}_{ftp}_{i}")
                          for i in range(2)
                      ]
                      wb = w_pool.tile([128, CT, 256], BF16, tag="w")
                      nc.gpsimd.dma_start(
                          out=wb[:],
                          in_=wqkv_d[
                              :, C + ftp * 256:C + (ftp + 1) * 256
                          ].rearrange("(co p) f -> p co f", p=128),
                      )
                      for ct in range(CT):
                          for i in range(2):
                              for j in range(0, N, 512):
                                  nc.tensor.matmul(
                                      ps_ks[i][:, j:j + 512],
                                      wb[:, ct, i * 128:(i + 1) * 128],
                                      xT[:, ct, j:j + 512],
                                      start=(ct == 0), stop=(ct == CT - 1),
                                  )
                      for i in range(2):
                          nc.vector.tensor_copy(
                              kT[:, 2 * ftp + i, 128:(MT) * 128],
                              ps_ks[i][:],
                          )

                  # ------- phase 2b: q^T f-tiles -> DRAM spill (reloaded by
                  # DMA in the attention loop; keeps PSUM free there)
                  for ftp in range(CT // 2):
                      ps_qs = [
                          psA.tile([128, N], F32, tag="psA",
                                   name=f"ps_q_{_rep}_{b}_{ftp}_{i}")
                          for i in range(2)
                      ]
                      wb = w_pool.tile([128, CT, 256], BF16, tag="w")
                      nc.gpsimd.dma_start(
                          out=wb[:],
                          in_=wqkv_d[
                              :, ftp * 256:(ftp + 1) * 256
                          ].rearrange("(co p) f -> p co f", p=128),
                      )
                      for ct in range(CT):
                          for i in range(2):
                              for j in range(0, N, 512):
                                  nc.tensor.matmul(
                                      ps_qs[i][:, j:j + 512],
                                      wb[:, ct, i * 128:(i + 1) * 128],
                                      xT[:, ct, j:j + 512],
                                      start=(ct == 0), stop=(ct == CT - 1),
                                  )
                      for i in range(2):
                          q_sb = o_pool.tile([128, N], BF16, tag="qsb")
                          nc.vector.tensor_copy(q_sb[:], ps_qs[i][:])
                          nc.sync.dma_start(
                              out=qsp_d[b, 2 * ftp + i], in_=q_sb[:]
                          )

                  # ------- phase 3: v_ext for ALL heads (BF16), out of the
                  # ACT-paced attention loop so PE prep overlaps nothing hot
                  vx_all = [None] * H
                  for hpp in range(HPAIRS // 2):  # two head-pairs per pass
                      ps_vs = [
                          psA.tile([128, N], F32, tag="psA",
                                   name=f"ps_v_{_rep}_{b}_{hpp}_{i}")
                          for i in range(2)
                      ]
                      wb = w_pool.tile([128, CT, 256], BF16, tag="w")
                      nc.gpsimd.dma_start(
                          out=wb[:],
                          in_=wqkv_d[
                              :, 2 * C + hpp * 256:2 * C + (hpp + 1) * 256
                          ].rearrange("(co p) f -> p co f", p=128),
                      )
                      for ct in range(CT):
                          for i in range(2):
                              for j in range(0, N, 512):
                                  nc.tensor.matmul(
                                      ps_vs[i][:, j:j + 512],
                                      wb[:, ct, i * 128:(i + 1) * 128],
                                      xT[:, ct, j:j + 512],
                                      start=(ct == 0), stop=(ct == CT - 1),
                                  )
                      for i in range(2):
                          hp = 2 * hpp + i
                          vTt = o_pool.tile([128, N], F32, tag="osb")
                          nc.vector.tensor_copy(vTt[:], ps_vs[i][:])
                          ps_tv = psB.tile([128, N], F32, tag="psB",
                                           name=f"ps_tv_{_rep}_{b}_{hp}")
                          for nt in range(NT):
                              nc.tensor.transpose(
                                  ps_tv[:, nt * 128:(nt + 1) * 128],
                                  vTt[:, nt * 128:(nt + 1) * 128],
                                  ident[:],
                              )
                          for hh in range(2):
                              h = 2 * hp + hh
                              vx = vx_pool.tile([128, MT, 128], BF16, tag=f"vx{h}",
                                                name=f"vx_{_rep}_{b}_{hp}_{hh}")
                              nc.vector.memset(vx[:, 0, :], 0.0)
                              nc.vector.memset(vx[:, 1:MT, 64:128], 1.0)
                              nc.vector.memset(vx[0:P, 0, 64:128], 1.0)
                              nc.gpsimd.dma_start(
                                  out=vx[0:P, 0, 0:64],
                                  in_=pv_d[b, :, h * D:(h + 1) * D],
                              )
                              nc.vector.tensor_copy(
                                  vx[:, 1:MT, 0:64],
                                  ps_tv[:].rearrange("p (a c) -> p a c", c=128)[
                                      :, :, hh * 64:(hh + 1) * 64
                                  ],
                              )
                              vx_all[h] = vx

                  # ---------------- phase 4: attention per head pair ---------
                  oT = big_pool.tile([128, CT, N], BF16, tag="big", name=f"oT_{_rep}_{b}")
                  for hp in range(HPAIRS):
                      vxs = [vx_all[2 * hp], vx_all[2 * hp + 1]]
                      qp = q_pool.tile([128, N], BF16, tag="qp")
                      nc.sync.dma_start(out=qp[:], in_=qsp_d[b, hp])

                      ps_av = [
                          psB.tile([128, N], F32, tag="psB",
                                   name=f"ps_av_{_rep}_{b}_{hp}_{i}")
                          for i in range(2)
                      ]
                      for mt in range(MT):
                          ps_ss = [
                              psA.tile([128, N], F32, tag="psA",
                                       name=f"ps_s_{_rep}_{b}_{hp}_{mt}_{i}")
                              for i in range(2)
                          ]
                          for j in range(0, N, 512):
                              for hh in range(2):
                                  base = hh * 64
                                  nc.tensor.matmul(
                                      ps_ss[hh][:, j:j + 512],
                                      kT[base:base + D, hp, mt * 128:(mt + 1) * 128],
                                      qp[base:base + D, j:j + 512],
                                      start=True, stop=True,
                                  )
                          eTs = []
                          for hh in range(2):
                              eTt = e_pool.tile([128, N], BF16, tag="eTb")
                              nc.scalar.activation(eTt[:], ps_ss[hh][:], AF.Exp,
                                                   scale=SCALE)
                              eTs.append(eTt)
                          for hh in range(2):
                              for j in range(0, N, 512):
                                  nc.tensor.matmul(
                                      ps_av[hh][:, j:j + 512],
                                      vxs[hh][:, mt, :],
                                      eTs[hh][:, j:j + 512],
                                      start=(mt == 0), stop=(mt == MT - 1),
                                  )
                      for hh in range(2):
                          # one copy frees the PSUM accumulator ASAP; the
                          # normalize then runs SBUF-only off the critical path
                          stg = o_pool.tile([128, N], F32, tag="osb")
                          nc.vector.tensor_copy(stg[:], ps_av[hh][:])
                          rb = rb_pool.tile([64, N], F32, tag="rb")
                          nc.vector.reciprocal(rb[:], stg[64:128, :])
                          nc.vector.tensor_mul(
                              oT[hh * 64:(hh + 1) * 64, hp, :],
                              stg[0:64, :],
                              rb[:],
                          )

                  # ---------------- phase 5: projection (transposed) ---------
                  for cfp in range(CT // 2):
                      ps_ps = [
                          psA.tile([128, N], F32, tag="psA",
                                   name=f"ps_p_{_rep}_{b}_{cfp}_{i}")
                          for i in range(2)
                      ]
                      wb = w_pool.tile([128, CT, 256], BF16, tag="w")
                      nc.gpsimd.dma_start(
                          out=wb[:],
                          in_=wproj_d[
                              :, cfp * 256:(cfp + 1) * 256
                          ].rearrange("(co p) f -> p co f", p=128),
                      )
                      for ct in range(CT):
                          for i in range(2):
                              for j in range(0, N, 512):
                                  nc.tensor.matmul(
                                      ps_ps[i][:, j:j + 512],
                                      wb[:, ct, i * 128:(i + 1) * 128],
                                      oT[:, ct, j:j + 512],
                                      start=(ct == 0), stop=(ct == CT - 1),
                                  )
                      for i in range(2):
                          cft = 2 * cfp + i
                          o_sb = o_pool.tile([128, N], F32, tag="osb")
                          nc.vector.tensor_scalar_add(
                              o_sb[:], ps_ps[i][:], bias_col[:, cft:cft + 1]
                          )
                          nc.sync.dma_start(
                              out=outT_d[b, cft * 128:(cft + 1) * 128, :],
                              in_=o_sb[:],
                          )

    return nc


_NC_CACHE = {}


def _get_nc(repeat: int = 1) -> bass.Bass:
    key = f"nc{repeat}"
    if key not in _NC_CACHE:
        _NC_CACHE[key] = build_nc(repeat)
    return _NC_CACHE[key]


def _make_runner(nc):
    """Compile the SPMD kernel ONCE into a reusable callable.

    Mirrors bass2jax.run_bass_via_pjrt's multi-core branch, but without
    output-buffer donation so the compiled function + device-resident
    inputs can be invoked repeatedly (for wall-clock benchmarking and to
    avoid recompiles on every kernel() call).
    """
    import jax
    from jax.experimental.shard_map import shard_map
    from jax.sharding import Mesh, PartitionSpec
    from concourse import bass2jax
    from concourse.bass2jax import _bass_exec_p, partition_id_tensor

    bass2jax.install_neuronx_cc_hook()

    partition_name = (
        nc.partition_id_tensor.name if nc.partition_id_tensor else None
    )
    in_names, out_names, out_avals, zero_outs = [], [], [], []
    for alloc in nc.m.functions[0].allocations:
        if not isinstance(alloc, mybir.MemoryLocationSet):
            continue
        name = alloc.memorylocations[0].name
        if alloc.kind == "ExternalInput":
            if name != partition_name:
                in_names.append(name)
        elif alloc.kind == "ExternalOutput":
            shape = tuple(alloc.tensor_shape)
            dtype = mybir.dt.np(alloc.dtype)
            out_names.append(name)
            out_avals.append(jax.core.ShapedArray(shape, dtype))
            zero_outs.append(np.zeros(shape, dtype))
    n_params = len(in_names)
    all_in_names = list(in_names) + list(out_names)
    if partition_name is not None:
        all_in_names.append(partition_name)

    def _body(*args):
        operands = list(args)
        if partition_name is not None:
            operands.append(partition_id_tensor())
        outs = _bass_exec_p.bind(
            *operands,
            out_avals=tuple(out_avals),
            in_names=tuple(all_in_names),
            out_names=tuple(out_names),
            lowering_input_output_aliases=(),
            sim_require_finite=True,
            sim_require_nnan=True,
            nc=nc,
        )
        return tuple(outs)

    devices = jax.devices()[:N_CORES]
    mesh = Mesh(np.asarray(devices), ("core",))
    n_outs = len(out_avals)
    in_specs = (PartitionSpec("core"),) * (n_params + n_outs)
    out_specs = (PartitionSpec("core"),) * n_outs
    sharded = jax.jit(
        shard_map(_body, mesh=mesh, in_specs=in_specs,
                  out_specs=out_specs, check_rep=False),
        keep_unused=True,
    )

    concat_zeros = [
        np.zeros((N_CORES * z.shape[0], *z.shape[1:]), z.dtype)
        for z in zero_outs
    ]

    state = {"dev_zeros": None}

    def runner(in_maps):
        per_core = [
            [np.asarray(m[name]) for name in in_names] for m in in_maps
        ]
        concat_in = [
            np.concatenate([per_core[c][i] for c in range(N_CORES)], axis=0)
            for i in range(n_params)
        ]
        if state["dev_zeros"] is None:
            state["dev_zeros"] = [jax.device_put(z) for z in concat_zeros]
        out_arrs = sharded(*concat_in, *state["dev_zeros"])
        return [
            {
                name: np.asarray(out_arrs[i]).reshape(
                    N_CORES, *out_avals[i].shape
                )[c]
                for i, name in enumerate(out_names)
            }
            for c in range(N_CORES)
        ]

    def runner_dev(dev_args):
        """dev_args: device-resident concat inputs; returns device outputs."""
        return sharded(*dev_args, *state["dev_zeros"])

    def make_dev_args(in_maps):
        per_core = [
            [np.asarray(m[name]) for name in in_names] for m in in_maps
        ]
        concat_in = [
            np.concatenate([per_core[c][i] for c in range(N_CORES)], axis=0)
            for i in range(n_params)
        ]
        if state["dev_zeros"] is None:
            state["dev_zeros"] = [jax.device_put(z) for z in concat_zeros]
        return [jax.device_put(a) for a in concat_in]

    return runner, runner_dev, make_dev_args


def _get_runner(repeat: int = 1):
    key = f"runner{repeat}"
    if key not in _NC_CACHE:
        _NC_CACHE[key] = _make_runner(_get_nc(repeat))
    return _NC_CACHE[key]


def _make_in_maps(x, pk, pv, w_qkv, w_proj, b_proj):
    x = np.ascontiguousarray(np.asarray(x, dtype=np.float32))
    pk = np.ascontiguousarray(np.asarray(pk, dtype=np.float32))
    pv = np.ascontiguousarray(np.asarray(pv, dtype=np.float32))
    w_qkv = np.ascontiguousarray(np.asarray(w_qkv, dtype=np.float32))
    w_proj = np.ascontiguousarray(np.asarray(w_proj, dtype=np.float32))
    b_proj = np.ascontiguousarray(np.asarray(b_proj, dtype=np.float32))
    in_maps = []
    for c in range(N_CORES):
        sl = slice(c * B_PC, (c + 1) * B_PC)
        in_maps.append({
            "x": x[sl], "pk": pk[sl], "pv": pv[sl],
            "w_qkv": w_qkv, "w_proj": w_proj, "b_proj": b_proj,
        })
    return in_maps


def run(x, pk, pv, w_qkv, w_proj, b_proj, trace=False, **trace_kwargs):
    """Run the SPMD kernel; returns (output [B,N,C], per-core results).

    With trace=True, runs through run_bass_kernel_spmd so the NTFF
    profile hook captures HW exec time; returns (out, BassKernelResults).
    """
    in_maps = _make_in_maps(x, pk, pv, w_qkv, w_proj, b_proj)
    if trace:
        res = run_bass_kernel_spmd(
            _get_nc(), in_maps, core_ids=list(range(N_CORES)),
            trace=True, **trace_kwargs,
        )
        results = res.results
    else:
        runner, _, _ = _get_runner()
        results = runner(in_maps)
        res = results
    out = np.empty((B, N, C), dtype=np.float32)
    for c in range(N_CORES):
        outT = results[c]["outT"]              # [B_PC, C, N]
        out[c * B_PC:(c + 1) * B_PC] = outT.transpose(0, 2, 1)
    return out, res


def kernel(x, pk, pv, w_qkv, w_proj, b_proj) -> np.ndarray:
    out, _ = run(x, pk, pv, w_qkv, w_proj, b_proj)
    return out


def benchmark(x, pk, pv, w_qkv, w_proj, b_proj, iters=20, warmup=3, repeat=1):
    """Median wall-clock per executed call with device-resident inputs."""
    import time
    import jax
    _, runner_dev, make_dev_args = _get_runner(repeat)
    in_maps = _make_in_maps(x, pk, pv, w_qkv, w_proj, b_proj)
    dev_args = make_dev_args(in_maps)
    for _ in range(warmup):
        outs = runner_dev(dev_args)
        jax.block_until_ready(outs)
    ts = []
    for _ in range(iters):
        t0 = time.perf_counter()
        outs = runner_dev(dev_args)
        jax.block_until_ready(outs)
        ts.append(time.perf_counter() - t0)
    ts.sort()
    return {
        "median_s": ts[len(ts) // 2],
        "min_s": ts[0],
        "all_s": ts,
    }

